# revision 5
# baseline (speedup 1.0000x reference)
"""NTM head addressing kernel for Trainium2 (8 NeuronCores, data-parallel over heads).

Shapes (hardcoded): B=4096 heads, N=2048 memory rows, C=128 memory cols.
Each core processes 512 heads as 4 tiles of 128 (partition dim = head).

Math restructuring vs the reference (exact up to fp rounding):
  - w = w_tilde^gamma / sum(w_tilde^gamma) is invariant to any per-head
    positive scale on w_tilde.  We therefore drop the softmax normalizer of
    s (divide taps by s1) and the (1-g) factor of the interpolation:
        u      = (g/(1-g)/sum_e) * e + w_prev          (e = exp(beta*sim))
        v      = (s0/s1)*u_{j-1} + u_j + (s2/s1)*u_{j+1}   (circular)
        w      = v^gamma / sum(v^gamma)
    with g/(1-g) = exp(g_raw) (sigmoid odds), s0/s1 = exp(s0_raw-s1_raw).
  - beta' = softplus(beta_raw)/||k|| is applied as the per-partition scale
    of the ACT exp pass reading the matmul PSUM; sum_e comes free via
    accum_out.  Likewise gamma' scales the final exp (accum_out=sum_y).
  - EPS terms are dropped: |denominators| >= ~2.5e-14 >> 1e-16 always.
"""

import os
import numpy as np

_B, _N, _C = 4096, 2048, 128
_NCORES = 8
_BS = _B // _NCORES      # 512 heads per core
_NT = _BS // 128         # 4 head tiles per core
_MT = _N // 128          # 16 memory-row tiles

# Intermediate dtype toggles (precision vs speed).
_CONV_BF16 = os.environ.get("NTM_CONV_BF16", "0") == "1"

_built = None


def _build():
    """Construct the (SPMD, per-core) Bass program. Returns (nc, input names)."""
    import concourse.bass as bass
    import concourse.bacc as bacc
    import concourse.mybir as mybir
    import concourse.tile as tile

    f32 = mybir.dt.float32
    bf16 = mybir.dt.bfloat16
    AF = mybir.ActivationFunctionType
    OP = mybir.AluOpType
    E = float(np.e)
    cdt = bf16 if _CONV_BF16 else f32

    nc = bacc.Bacc(
        "TRN2", target_bir_lowering=False, debug=False, num_devices=_NCORES
    )
    kT_d = nc.declare_dram_parameter("kT", [_C, _BS], f32, isOutput=False)
    kR_d = nc.declare_dram_parameter("kR", [_BS, _C], f32, isOutput=False)
    sm_d = nc.declare_dram_parameter("sm", [128, _NT * 6], f32, isOutput=False)
    wp_d = nc.declare_dram_parameter("wp", [_BS, _N], f32, isOutput=False)
    M_d = nc.declare_dram_parameter("M", [_N, _C], f32, isOutput=False)
    eye_d = nc.declare_dram_parameter("eye", [128, 128], f32, isOutput=False)
    out_d = nc.declare_dram_parameter("out", [_BS, _N], f32, isOutput=True)

    with tile.TileContext(nc) as tc:
        with (
            tc.tile_pool(name="const", bufs=1) as constp,
            tc.tile_pool(name="setup", bufs=1) as setupp,
            tc.tile_pool(name="slab", bufs=2) as slabp,
            tc.tile_pool(name="mini", bufs=2) as minip,
            tc.tile_pool(name="psum_mm", bufs=1, space=bass.MemorySpace.PSUM) as psmm,
            tc.tile_pool(name="psum_tr", bufs=2, space=bass.MemorySpace.PSUM) as pstr,
        ):
            # ---------------- setup: loads ----------------
            eye = constp.tile([128, 128], f32)
            nc.sync.dma_start(eye[:], eye_d[:])
            kT = constp.tile([_C, _BS], f32)
            nc.sync.dma_start(kT[:], kT_d[:])
            kR = constp.tile([128, _NT, _C], f32)
            nc.sync.dma_start(kR[:], kR_d[:].rearrange("(t p) c -> p t c", p=128))
            sm = constp.tile([128, _NT * 6], f32)
            nc.sync.dma_start(sm[:], sm_d[:])
            mrows = constp.tile([128, _MT, _C], f32)
            nc.sync.dma_start(mrows[:], M_d[:].rearrange("(t p) c -> p t c", p=128))

            # ---------------- setup: M row norms -> normalized M^T ----------
            # msq_sq = M*M on GPSIMD (frees DVE); row-sums on DVE.
            msq_sq = setupp.tile([128, _MT, _C], f32)
            nc.gpsimd.tensor_mul(msq_sq[:], mrows[:], mrows[:])
            msq = minip.tile([128, _MT], f32, tag="msq")
            nc.vector.tensor_reduce(msq[:], msq_sq[:], mybir.AxisListType.X, OP.add)
            # rmn = msq^-0.5 = exp(-0.5*ln(msq))  (ACT Rsqrt is banned)
            lnmsq = minip.tile([128, _MT], f32, tag="lnmsq")
            nc.scalar.activation(lnmsq[:], msq[:], AF.Ln)
            rmn = minip.tile([128, _MT], f32, tag="rmn")
            nc.scalar.activation(rmn[:], lnmsq[:], AF.Exp, scale=-0.5)

            mhat = setupp.tile([128, _MT, _C], f32)
            MTh = constp.tile([_C, _N], f32)
            for mt in range(_MT):
                nc.vector.tensor_scalar_mul(
                    mhat[:, mt, :], mrows[:, mt, :], rmn[:, mt : mt + 1]
                )
                trp = pstr.tile([128, 128], f32)
                nc.tensor.transpose(trp[:], mhat[:, mt, :], eye[:])
                nc.scalar.copy(MTh[:, mt * 128 : (mt + 1) * 128], trp[:])

            # ---------------- setup: per-head scalars ----------------
            # ksq_t = sum_c k^2 (per head): square then row-reduce
            ksq_sq = setupp.tile([128, _NT, _C], f32)
            nc.vector.tensor_mul(ksq_sq[:], kR[:], kR[:])
            ksq = minip.tile([128, _NT], f32, tag="ksq")
            nc.vector.tensor_reduce(ksq[:], ksq_sq[:], mybir.AxisListType.X, OP.add)
            lnksq = minip.tile([128, _NT], f32, tag="lnksq")
            nc.scalar.activation(lnksq[:], ksq[:], AF.Ln)
            rk = minip.tile([128, _NT], f32, tag="rk")
            nc.scalar.activation(rk[:], lnksq[:], AF.Exp, scale=-0.5)

            # beta' = softplus(beta_raw) * rk ; softplus = ln(1 + exp(x))
            be = minip.tile([128, _NT], f32, tag="be")
            nc.scalar.activation(be[:], sm[:, 0:_NT], AF.Exp)
            bsp = minip.tile([128, _NT], f32, tag="bsp")
            nc.scalar.activation(bsp[:], be[:], AF.Ln, bias=1.0)
            bprime = minip.tile([128, _NT], f32, tag="bprime")
            nc.vector.tensor_mul(bprime[:], bsp[:], rk[:])

            # eg = exp(g_raw) = g/(1-g)
            eg = minip.tile([128, _NT], f32, tag="eg")
            nc.scalar.activation(eg[:], sm[:, _NT : 2 * _NT], AF.Exp)

            # gamma' = 1 + softplus(gamma_raw)
            ge = minip.tile([128, _NT], f32, tag="ge")
            nc.scalar.activation(ge[:], sm[:, 2 * _NT : 3 * _NT], AF.Exp)
            gsp = minip.tile([128, _NT], f32, tag="gsp")
            nc.scalar.activation(gsp[:], ge[:], AF.Ln, bias=1.0)
            gprime = minip.tile([128, _NT], f32, tag="gprime")
            nc.vector.tensor_scalar_add(gprime[:], gsp[:], 1.0)

            # conv taps: s0' = exp(s0-s1), s2' = exp(s2-s1) -> s02 [128, 2*NT]
            d02 = minip.tile([128, 2 * _NT], f32, tag="d02")
            nc.vector.tensor_sub(d02[:, 0:_NT], sm[:, 3 * _NT : 4 * _NT], sm[:, 4 * _NT : 5 * _NT])
            nc.vector.tensor_sub(d02[:, _NT : 2 * _NT], sm[:, 5 * _NT : 6 * _NT], sm[:, 4 * _NT : 5 * _NT])
            s02 = minip.tile([128, 2 * _NT], f32, tag="s02")
            nc.scalar.activation(s02[:], d02[:], AF.Exp)

            # ---------------- main loop over 4 head tiles ----------------
            for t in range(_NT):
                wp = slabp.tile([128, _N], f32, tag="wp")
                nc.sync.dma_start(wp[:], wp_d[:][t * 128 : (t + 1) * 128, :])

                logits = psmm.tile([128, _N], f32, tag="logits")
                for j in range(_N // 512):
                    nc.tensor.matmul(
                        logits[:, j * 512 : (j + 1) * 512],
                        kT[:, t * 128 : (t + 1) * 128],
                        MTh[:, j * 512 : (j + 1) * 512],
                    )

                # e = exp(beta' * logits), sum_e fused
                e = slabp.tile([128, _N], f32, tag="e")
                sume = minip.tile([128, 1], f32, tag="sume")
                nc.scalar.activation(
                    e[:], logits[:], AF.Exp,
                    scale=bprime[:, t : t + 1], accum_out=sume[:],
                )

                # a = eg / sum_e
                rse = minip.tile([128, 1], f32, tag="rse")
                nc.vector.reciprocal(rse[:], sume[:])
                a = minip.tile([128, 1], f32, tag="a")
                nc.vector.tensor_mul(a[:], eg[:, t : t + 1], rse[:])

                # u = a*e + w_prev
                u = slabp.tile([128, _N], cdt, tag="u")
                nc.vector.scalar_tensor_tensor(
                    u[:], e[:], a[:], wp[:], OP.mult, OP.add
                )

                # circular 3-tap conv, middle tap 1:
                #   c = s0'*u_{j-1} + u_j ;  v = s2'*u_{j+1} + c
                s0a = s02[:, t : t + 1]
                s2a = s02[:, _NT + t : _NT + t + 1]
                c = slabp.tile([128, _N], cdt, tag="c")
                nc.vector.scalar_tensor_tensor(
                    c[:, 1:_N], u[:, 0 : _N - 1], s0a, u[:, 1:_N], OP.mult, OP.add
                )
                nc.vector.scalar_tensor_tensor(
                    c[:, 0:1], u[:, _N - 1 : _N], s0a, u[:, 0:1], OP.mult, OP.add
                )
                v = slabp.tile([128, _N], cdt, tag="v")
                nc.vector.scalar_tensor_tensor(
                    v[:, 0 : _N - 1], u[:, 1:_N], s2a, c[:, 0 : _N - 1], OP.mult, OP.add
                )
                nc.vector.scalar_tensor_tensor(
                    v[:, _N - 1 : _N], u[:, 0:1], s2a, c[:, _N - 1 : _N], OP.mult, OP.add
                )

                # sharpen: y = v^gamma' = exp(gamma' * ln v), sum_y fused
                lw = slabp.tile([128, _N], f32, tag="lw")
                nc.scalar.activation(lw[:], v[:], AF.Ln)
                y = slabp.tile([128, _N], f32, tag="y")
                sumy = minip.tile([128, 1], f32, tag="sumy")
                nc.scalar.activation(
                    y[:], lw[:], AF.Exp,
                    scale=gprime[:, t : t + 1], accum_out=sumy[:],
                )

                # w = y / sum_y  (scale pass on GPSIMD: contention-free vs
                # DVE 1x-mode STT ops, and ACT is the busier engine)
                r = minip.tile([128, 1], f32, tag="r")
                nc.vector.reciprocal(r[:], sumy[:])
                wout = slabp.tile([128, _N], f32, tag="wout")
                nc.gpsimd.tensor_scalar_mul(wout[:], y[:], r[:])

                nc.sync.dma_start(out_d[:][t * 128 : (t + 1) * 128, :], wout[:])

    nc.compile()
    return nc


def _get_nc():
    global _built
    if _built is None:
        _built = _build()
    return _built


def _make_in_maps(k, beta, g, s, gamma, w_prev, M):
    eye = np.eye(128, dtype=np.float32)
    Mc = np.ascontiguousarray(M, dtype=np.float32)
    in_maps = []
    for c in range(_NCORES):
        sl = slice(c * _BS, (c + 1) * _BS)
        ks = np.ascontiguousarray(k[sl], dtype=np.float32)          # [512,128]
        kTs = np.ascontiguousarray(ks.T)                            # [128,512]
        # packed per-head scalars: [128, 6*NT]; col block order:
        # beta, g, gamma, s0, s1, s2 (each NT wide; head = t*128 + p)
        def cols(x):
            return np.ascontiguousarray(x.reshape(_NT, 128).T, dtype=np.float32)
        sm = np.concatenate(
            [
                cols(beta[sl, 0]),
                cols(g[sl, 0]),
                cols(gamma[sl, 0]),
                cols(s[sl, 0]),
                cols(s[sl, 1]),
                cols(s[sl, 2]),
            ],
            axis=1,
        )
        in_maps.append(
            {
                "kT": kTs,
                "kR": ks,
                "sm": np.ascontiguousarray(sm),
                "wp": np.ascontiguousarray(w_prev[sl], dtype=np.float32),
                "M": Mc,
                "eye": eye,
            }
        )
    return in_maps


def kernel(k, beta, g, s, gamma, w_prev, M, _trace=False, _tmpdir=None):
    from concourse.bass_utils import run_bass_kernel_spmd

    nc = _get_nc()
    in_maps = _make_in_maps(
        np.asarray(k), np.asarray(beta), np.asarray(g), np.asarray(s),
        np.asarray(gamma), np.asarray(w_prev), np.asarray(M),
    )
    res = run_bass_kernel_spmd(
        nc, in_maps, list(range(_NCORES)), trace=_trace, tmpdir=_tmpdir
    )
    out = np.concatenate([res.results[c]["out"] for c in range(_NCORES)], axis=0)
    if _trace:
        kernel._last_results = res
    return out


# revision 6
# speedup vs baseline: 2.3897x; 2.3897x over previous
"""NTM head addressing kernel for Trainium2 (8 NeuronCores, data-parallel over heads).

Shapes (hardcoded): B=4096 heads, N=2048 memory rows, C=128 memory cols.
Each core processes 512 heads as 4 tiles of 128 (partition dim = head).

Math restructuring vs the reference (exact up to fp rounding):
  - w = w_tilde^gamma / sum(w_tilde^gamma) is invariant to any per-head
    positive scale on w_tilde.  We therefore drop the softmax normalizer of
    s (divide taps by s1) and the (1-g) factor of the interpolation:
        u      = (g/(1-g)/sum_e) * e + w_prev          (e = exp(beta*sim))
        v      = (s0/s1)*u_{j-1} + u_j + (s2/s1)*u_{j+1}   (circular)
        w      = v^gamma / sum(v^gamma)
    with g/(1-g) = exp(g_raw) (sigmoid odds), s0/s1 = exp(s0_raw-s1_raw).
  - beta' = softplus(beta_raw)/||k|| is applied as the per-partition scale
    of the ACT exp pass reading the matmul PSUM; sum_e comes free via
    accum_out.  Likewise gamma' scales the final exp (accum_out=sum_y).
  - EPS terms are dropped: |denominators| >= ~2.5e-14 >> 1e-16 always.

Engine budget per core (all activation functions pinned to the single
natural_log_exp_and_others table set -> one ACT_TABLE_LOAD total):
  DVE: 3 scalar_tensor_tensor passes per tile (interp + 2 conv taps),
       narrow circular-edge STTs, half the setup scales/copies.
  ACT: exp / ln / exp(gamma*..) passes (+fused row sums), half the setup.
  PE:  fp32 matmul (sim) + 16 M^T transposes.  GPSIMD: M square only.
"""

import os
import numpy as np

_B, _N, _C = 4096, 2048, 128
_NCORES = 8
_BS = _B // _NCORES      # 512 heads per core
_NT = _BS // 128         # 4 head tiles per core
_MT = _N // 128          # 16 memory-row tiles

_built = None

_ONE_SET = "natural_log_exp_and_others"
_PINNED = {"Exp", "Ln", "Square", "Copy", "Identity"}


def _patch_act_tables():
    """Force Exp/Ln/Square/Copy onto the one table set that holds them all,
    so bacc's load inserter cannot thrash between per-function sets."""
    import concourse.bacc as bacc
    import concourse.hw_specs as hw_specs
    import concourse.mybir as mybir

    if getattr(bacc, "_ntm_table_patch", False):
        return
    orig = hw_specs.get_activation_tables
    pinned = {
        getattr(mybir.ActivationFunctionType, n)
        for n in _PINNED
        if hasattr(mybir.ActivationFunctionType, n)
    }

    def patched(module_arch):
        tables = orig(module_arch)
        out = {}
        for name, fns in tables.items():
            if name != _ONE_SET:
                fns = fns - pinned
            out[name] = fns
        return out

    bacc.get_activation_tables = patched
    bacc._ntm_table_patch = True


def _build():
    """Construct the (SPMD, per-core) Bass program."""
    import concourse.bass as bass
    import concourse.bacc as bacc
    import concourse.mybir as mybir
    import concourse.tile as tile

    _patch_act_tables()

    f32 = mybir.dt.float32
    AF = mybir.ActivationFunctionType
    OP = mybir.AluOpType

    nc = bacc.Bacc(
        "TRN2", target_bir_lowering=False, debug=False, num_devices=_NCORES
    )
    kT_d = nc.declare_dram_parameter("kT", [_C, _BS], f32, isOutput=False)
    kR_d = nc.declare_dram_parameter("kR", [_BS, _C], f32, isOutput=False)
    sm_d = nc.declare_dram_parameter("sm", [128, _NT * 6], f32, isOutput=False)
    wp_d = nc.declare_dram_parameter("wp", [_BS, _N], f32, isOutput=False)
    M_d = nc.declare_dram_parameter("M", [_N, _C], f32, isOutput=False)
    eye_d = nc.declare_dram_parameter("eye", [128, 128], f32, isOutput=False)
    out_d = nc.declare_dram_parameter("out", [_BS, _N], f32, isOutput=True)

    with tile.TileContext(nc) as tc:
        with (
            tc.tile_pool(name="const", bufs=1) as constp,
            tc.tile_pool(name="setup", bufs=1) as setupp,
            tc.tile_pool(name="slab", bufs=2) as slabp,
            tc.tile_pool(name="mini", bufs=2) as minip,
            tc.tile_pool(name="psum", bufs=2, space=bass.MemorySpace.PSUM) as psump,
        ):
            # ---------------- setup: loads ----------------
            eye = constp.tile([128, 128], f32)
            nc.sync.dma_start(eye[:], eye_d[:])
            kT = constp.tile([_C, _BS], f32)
            nc.sync.dma_start(kT[:], kT_d[:])
            kR = constp.tile([128, _NT, _C], f32)
            nc.sync.dma_start(kR[:], kR_d[:].rearrange("(t p) c -> p t c", p=128))
            sm = constp.tile([128, _NT * 6], f32)
            nc.sync.dma_start(sm[:], sm_d[:])
            mrows = constp.tile([128, _MT, _C], f32)
            nc.sync.dma_start(mrows[:], M_d[:].rearrange("(t p) c -> p t c", p=128))

            # ---------------- setup: M row norms -> normalized M^T ----------
            # square on GPSIMD (otherwise idle), row-sums on DVE
            msq_sq = setupp.tile([128, _MT, _C], f32)
            nc.gpsimd.tensor_mul(msq_sq[:], mrows[:], mrows[:])
            msq = minip.tile([128, _MT], f32, tag="msq")
            nc.vector.tensor_reduce(msq[:], msq_sq[:], mybir.AxisListType.X, OP.add)
            # rmn = msq^-0.5 = exp(-0.5*ln(msq))  (ACT Rsqrt is banned)
            lnmsq = minip.tile([128, _MT], f32, tag="lnmsq")
            nc.scalar.activation(lnmsq[:], msq[:], AF.Ln)
            rmn = minip.tile([128, _MT], f32, tag="rmn")
            nc.scalar.activation(rmn[:], lnmsq[:], AF.Exp, scale=-0.5)

            mhat = setupp.tile([128, _MT, _C], f32)
            MTh = constp.tile([_C, _N], f32)
            for mt in range(_MT):
                # alternate setup work between DVE and ACT
                if mt % 2 == 0:
                    nc.vector.tensor_scalar_mul(
                        mhat[:, mt, :], mrows[:, mt, :], rmn[:, mt : mt + 1]
                    )
                else:
                    nc.scalar.mul(
                        mhat[:, mt, :], mrows[:, mt, :], rmn[:, mt : mt + 1]
                    )
                trp = psump.tile([128, _N], f32, tag="ps")
                nc.tensor.transpose(trp[:, 0:128], mhat[:, mt, :], eye[:])
                if mt % 2 == 0:
                    nc.scalar.copy(MTh[:, mt * 128 : (mt + 1) * 128], trp[:, 0:128])
                else:
                    nc.vector.tensor_copy(
                        MTh[:, mt * 128 : (mt + 1) * 128], trp[:, 0:128]
                    )

            # ---------------- setup: per-head scalars ----------------
            # ksq_t = sum_c k^2 via fused ACT Square + accumulate
            ksq = minip.tile([128, _NT], f32, tag="ksq")
            kscr = setupp.tile([128, _C], f32)
            for t in range(_NT):
                nc.scalar.activation(
                    kscr[:], kR[:, t, :], AF.Square, accum_out=ksq[:, t : t + 1]
                )
            lnksq = minip.tile([128, _NT], f32, tag="lnksq")
            nc.scalar.activation(lnksq[:], ksq[:], AF.Ln)
            rk = minip.tile([128, _NT], f32, tag="rk")
            nc.scalar.activation(rk[:], lnksq[:], AF.Exp, scale=-0.5)

            # beta' = softplus(beta_raw) * rk ; softplus = ln(1 + exp(x))
            be = minip.tile([128, _NT], f32, tag="be")
            nc.scalar.activation(be[:], sm[:, 0:_NT], AF.Exp)
            bsp = minip.tile([128, _NT], f32, tag="bsp")
            nc.scalar.activation(bsp[:], be[:], AF.Ln, bias=1.0)
            bprime = minip.tile([128, _NT], f32, tag="bprime")
            nc.vector.tensor_mul(bprime[:], bsp[:], rk[:])

            # eg = exp(g_raw) = g/(1-g)
            eg = minip.tile([128, _NT], f32, tag="eg")
            nc.scalar.activation(eg[:], sm[:, _NT : 2 * _NT], AF.Exp)

            # gamma' = 1 + softplus(gamma_raw)
            ge = minip.tile([128, _NT], f32, tag="ge")
            nc.scalar.activation(ge[:], sm[:, 2 * _NT : 3 * _NT], AF.Exp)
            gsp = minip.tile([128, _NT], f32, tag="gsp")
            nc.scalar.activation(gsp[:], ge[:], AF.Ln, bias=1.0)
            gprime = minip.tile([128, _NT], f32, tag="gprime")
            nc.vector.tensor_scalar_add(gprime[:], gsp[:], 1.0)

            # conv taps: s0' = exp(s0-s1), s2' = exp(s2-s1) -> s02 [128, 2*NT]
            d02 = minip.tile([128, 2 * _NT], f32, tag="d02")
            nc.vector.tensor_sub(d02[:, 0:_NT], sm[:, 3 * _NT : 4 * _NT], sm[:, 4 * _NT : 5 * _NT])
            nc.vector.tensor_sub(d02[:, _NT : 2 * _NT], sm[:, 5 * _NT : 6 * _NT], sm[:, 4 * _NT : 5 * _NT])
            s02 = minip.tile([128, 2 * _NT], f32, tag="s02")
            nc.scalar.activation(s02[:], d02[:], AF.Exp)

            # ---------------- main loop over 4 head tiles ----------------
            for t in range(_NT):
                wp = slabp.tile([128, _N], f32, tag="wp")
                nc.sync.dma_start(wp[:], wp_d[:][t * 128 : (t + 1) * 128, :])

                logits = psump.tile([128, _N], f32, tag="ps")
                for j in range(_N // 512):
                    nc.tensor.matmul(
                        logits[:, j * 512 : (j + 1) * 512],
                        kT[:, t * 128 : (t + 1) * 128],
                        MTh[:, j * 512 : (j + 1) * 512],
                    )

                # e = exp(beta' * logits), sum_e fused
                e = slabp.tile([128, _N], f32, tag="e")
                sume = minip.tile([128, 1], f32, tag="sume")
                nc.scalar.activation(
                    e[:], logits[:], AF.Exp,
                    scale=bprime[:, t : t + 1], accum_out=sume[:],
                )

                # a = eg / sum_e
                rse = minip.tile([128, 1], f32, tag="rse")
                nc.vector.reciprocal(rse[:], sume[:])
                a = minip.tile([128, 1], f32, tag="a")
                nc.vector.tensor_mul(a[:], eg[:, t : t + 1], rse[:])

                # u = a*e + w_prev
                u = slabp.tile([128, _N], f32, tag="u")
                nc.vector.scalar_tensor_tensor(
                    u[:], e[:], a[:], wp[:], OP.mult, OP.add
                )

                # circular 3-tap conv, middle tap 1:
                #   c = s0'*u_{j-1} + u_j ;  v = s2'*u_{j+1} + c
                s0a = s02[:, t : t + 1]
                s2a = s02[:, _NT + t : _NT + t + 1]
                c = slabp.tile([128, _N], f32, tag="c")
                nc.vector.scalar_tensor_tensor(
                    c[:, 1:_N], u[:, 0 : _N - 1], s0a, u[:, 1:_N], OP.mult, OP.add
                )
                nc.vector.scalar_tensor_tensor(
                    c[:, 0:1], u[:, _N - 1 : _N], s0a, u[:, 0:1], OP.mult, OP.add
                )
                v = slabp.tile([128, _N], f32, tag="v")
                nc.vector.scalar_tensor_tensor(
                    v[:, 0 : _N - 1], u[:, 1:_N], s2a, c[:, 0 : _N - 1], OP.mult, OP.add
                )
                nc.vector.scalar_tensor_tensor(
                    v[:, _N - 1 : _N], u[:, 0:1], s2a, c[:, _N - 1 : _N], OP.mult, OP.add
                )

                # sharpen: y = v^gamma' = exp(gamma' * ln v), sum_y fused
                lw = slabp.tile([128, _N], f32, tag="lw")
                nc.scalar.activation(lw[:], v[:], AF.Ln)
                y = slabp.tile([128, _N], f32, tag="y")
                sumy = minip.tile([128, 1], f32, tag="sumy")
                nc.scalar.activation(
                    y[:], lw[:], AF.Exp,
                    scale=gprime[:, t : t + 1], accum_out=sumy[:],
                )

                # w = y / sum_y  (alternate final pass between ACT and DVE)
                r = minip.tile([128, 1], f32, tag="r")
                nc.vector.reciprocal(r[:], sumy[:])
                wout = slabp.tile([128, _N], f32, tag="wout")
                if t % 2 == 0:
                    nc.scalar.mul(wout[:], y[:], r[:])
                else:
                    nc.vector.tensor_scalar_mul(wout[:], y[:], r[:])

                nc.sync.dma_start(out_d[:][t * 128 : (t + 1) * 128, :], wout[:])

    nc.compile()
    return nc


def _get_nc():
    global _built
    if _built is None:
        _built = _build()
    return _built


def _make_in_maps(k, beta, g, s, gamma, w_prev, M):
    eye = np.eye(128, dtype=np.float32)
    Mc = np.ascontiguousarray(M, dtype=np.float32)
    in_maps = []
    for c in range(_NCORES):
        sl = slice(c * _BS, (c + 1) * _BS)
        ks = np.ascontiguousarray(k[sl], dtype=np.float32)          # [512,128]
        kTs = np.ascontiguousarray(ks.T)                            # [128,512]
        # packed per-head scalars: [128, 6*NT]; col block order:
        # beta, g, gamma, s0, s1, s2 (each NT wide; head = t*128 + p)
        def cols(x):
            return np.ascontiguousarray(x.reshape(_NT, 128).T, dtype=np.float32)
        sm = np.concatenate(
            [
                cols(beta[sl, 0]),
                cols(g[sl, 0]),
                cols(gamma[sl, 0]),
                cols(s[sl, 0]),
                cols(s[sl, 1]),
                cols(s[sl, 2]),
            ],
            axis=1,
        )
        in_maps.append(
            {
                "kT": kTs,
                "kR": ks,
                "sm": np.ascontiguousarray(sm),
                "wp": np.ascontiguousarray(w_prev[sl], dtype=np.float32),
                "M": Mc,
                "eye": eye,
            }
        )
    return in_maps


def kernel(k, beta, g, s, gamma, w_prev, M, _trace=False, _tmpdir=None):
    from concourse.bass_utils import run_bass_kernel_spmd

    nc = _get_nc()
    in_maps = _make_in_maps(
        np.asarray(k), np.asarray(beta), np.asarray(g), np.asarray(s),
        np.asarray(gamma), np.asarray(w_prev), np.asarray(M),
    )
    res = run_bass_kernel_spmd(
        nc, in_maps, list(range(_NCORES)), trace=_trace, tmpdir=_tmpdir
    )
    out = np.concatenate([res.results[c]["out"] for c in range(_NCORES)], axis=0)
    if _trace:
        kernel._last_results = res
    return out
